# revision 1
# baseline (speedup 1.0000x reference)
"""TransformerConv (heads=1) + ELU layer as a Bass/Tile kernel on 8 NeuronCores.

Strategy (1D graph partition by target node):
  - dst nodes sharded 8 ways (12500/core, padded to 98 blocks x 128).
  - Each core redundantly computes k=x@Wk, v=x@Wv for ALL nodes (bf16) into a
    DRAM table with k|v interleaved per row (512B rows -> full-efficiency
    indirect gathers), plus q'=x@Wq+bq and skip=x@Ws+(bs+bv) for its dst slice.
    The k bias cancels inside the per-dst segment softmax; the v bias sums to
    bv (sum alpha = 1) and is folded into the skip bias.
  - Edges are bucketed by dst block on the host; per 128-edge chunk the core
    gathers k|v rows by src and q rows by dst (indirect DMA), computes
    logits = rowsum(qg*kg)*scale (fused tensor_tensor_reduce), ex = exp(logit)
    (no max subtraction - logits are O(1) for this data), builds an ex-weighted
    one-hot matrix M[e,d] = ex_e * (dstloc_e == d), and uses the TensorEngine
    to scatter-add: agg += M^T @ Vg, den += M^T @ 1.
  - Epilogue per block: out = elu(agg/den + skip), streamed to DRAM.
Pad slots gather row 0 (real data) and have dstloc=255 so their one-hot row is
zero - they contribute nothing.
"""
import math
import numpy as np
import ml_dtypes

BF16 = ml_dtypes.bfloat16

N, E, D = 100000, 800000, 128
M_CORES = 8
DPC = N // M_CORES                 # 12500
NB = (DPC + 127) // 128            # 98
DST_PAD = NB * 128                 # 12544
NPAD = ((N + 127) // 128) * 128    # 100096
SCALE = 1.0 / math.sqrt(D)
TW = 2048                          # phase-1 row-tile width
BG = 7                             # dst blocks per gather group (98 = 14*7)


def _host_prep(edge_index):
    """Pack dsts into blocks against a shared per-block chunk-count profile.

    Returns (idx_kv [M,128,S], dstloc [M,128,S], perm [M,DST_PAD], cc tuple)
    where S = sum(cc); perm[c][device_row] = local dst id (or >= DPC for pads).
    """
    src = np.asarray(edge_index[0], dtype=np.int64)
    dst = np.asarray(edge_index[1], dtype=np.int64)
    core = dst // DPC
    ld = dst - core * DPC

    deg = np.zeros((M_CORES, DST_PAD), np.int64)
    for c in range(M_CORES):
        deg[c, :DPC] = np.bincount(ld[core == c], minlength=DPC)[:DPC]
    edges_per_core = deg.sum(axis=1)

    slack = 384
    k9 = int(max(0, np.ceil((edges_per_core.max() + slack - NB * 1024) / 128.0)))
    while True:
        cc = np.array([9] * k9 + [8] * (NB - k9), np.int64)
        caps = cc * 128
        # batched LPT against capacities: 128 batches of 98 items
        assign = np.zeros((M_CORES, DST_PAD), np.int64)  # local dst -> block
        ok = True
        for c in range(M_CORES):
            order = np.argsort(-deg[c])
            loads = np.zeros(NB, np.int64)
            for k in range(128):
                batch = order[k * NB:(k + 1) * NB]
                binord = np.argsort(loads - caps)
                assign[c, batch] = binord
                loads[binord] += deg[c, batch]
            if (loads > caps).any():
                ok = False
                break
        if ok:
            break
        k9 += 1
    S = int(cc.sum())
    colbase = np.concatenate([[0], np.cumsum(cc)[:-1]])

    idx_kv = np.zeros((M_CORES, 128, S), np.int32)
    dstloc = np.full((M_CORES, 128, S), 255.0, np.float32)
    perm = np.zeros((M_CORES, DST_PAD), np.int64)
    for c in range(M_CORES):
        blk = assign[c]                      # local dst -> block
        # lane of each dst within its block (order of appearance)
        order = np.argsort(blk, kind="stable")
        blk_sorted = blk[order]
        starts = np.searchsorted(blk_sorted, np.arange(NB))
        lane = np.arange(DST_PAD) - starts[blk_sorted]
        # device row of dst order[i] = blk_sorted[i]*128 + lane[i]
        rows = blk_sorted * 128 + lane
        perm[c, rows] = order                # device row -> local dst
        lane_of = np.zeros(DST_PAD, np.int64)
        lane_of[order] = lane

        sel = core == c
        e_ld = ld[sel]
        e_src = src[sel]
        e_blk = blk[e_ld]
        g_order = np.argsort(e_blk, kind="stable")
        gb = e_blk[g_order]
        counts = np.bincount(gb, minlength=NB)
        if (counts > caps).any():
            raise RuntimeError("packing overflow")
        estarts = np.concatenate([[0], np.cumsum(counts)[:-1]])
        j = np.arange(len(gb)) - estarts[gb]
        c_of = j // 128
        p_of = j % 128
        scol = colbase[gb] + c_of
        idx_kv[c, p_of, scol] = e_src[g_order].astype(np.int32)
        dstloc[c, p_of, scol] = lane_of[e_ld[g_order]].astype(np.float32)
    return idx_kv, dstloc, perm, tuple(int(x) for x in cc)


def _build_nc(cc, npad=NPAD, dst_pad=DST_PAD, tw=TW):
    from contextlib import ExitStack
    import concourse.bass as bass
    import concourse.tile as tile
    from concourse import bacc, mybir

    fp32 = mybir.dt.float32
    bf16 = mybir.dt.bfloat16
    i32 = mybir.dt.int32
    Alu = mybir.AluOpType
    Act = mybir.ActivationFunctionType

    nc = bacc.Bacc("TRN2", target_bir_lowering=False, debug=False)
    nb = len(cc)
    S = int(sum(cc))
    colbase = [0]
    for x in cc[:-1]:
        colbase.append(colbase[-1] + x)

    xT = nc.dram_tensor("xT", [128, npad], bf16, kind="ExternalInput").ap()
    xTs = nc.dram_tensor("xTs", [128, dst_pad], bf16, kind="ExternalInput").ap()
    Wq = nc.dram_tensor("Wq", [128, 128], bf16, kind="ExternalInput").ap()
    Wk = nc.dram_tensor("Wk", [128, 128], bf16, kind="ExternalInput").ap()
    Wv = nc.dram_tensor("Wv", [128, 128], bf16, kind="ExternalInput").ap()
    Ws = nc.dram_tensor("Ws", [128, 128], bf16, kind="ExternalInput").ap()
    bq1 = nc.dram_tensor("bq1", [1, 128], bf16, kind="ExternalInput").ap()
    bsv1 = nc.dram_tensor("bsv1", [1, 128], bf16, kind="ExternalInput").ap()
    idx_kv_d = nc.dram_tensor("idx_kv", [128, S], i32, kind="ExternalInput").ap()
    dstloc_d = nc.dram_tensor("dstloc", [128, S], fp32, kind="ExternalInput").ap()

    kv_tab = nc.dram_tensor("kv_tab", [npad, 256], bf16, kind="Internal").ap()
    out_d = nc.dram_tensor("out", [dst_pad, 128], fp32, kind="ExternalOutput").ap()

    with tile.TileContext(nc) as tc, ExitStack() as ctx:
        const_p = ctx.enter_context(tc.tile_pool(name="const", bufs=1))

        # constants
        w_q = const_p.tile([128, 128], bf16, tag="wq")
        w_k = const_p.tile([128, 128], bf16, tag="wk")
        w_v = const_p.tile([128, 128], bf16, tag="wv")
        w_s = const_p.tile([128, 128], bf16, tag="ws")
        b_q = const_p.tile([1, 128], bf16, tag="bq")
        b_sv = const_p.tile([1, 128], bf16, tag="bsv")
        nc.sync.dma_start(w_q[:], Wq[:])
        nc.sync.dma_start(w_k[:], Wk[:])
        nc.sync.dma_start(w_v[:], Wv[:])
        nc.sync.dma_start(w_s[:], Ws[:])
        nc.sync.dma_start(b_q[:], bq1[:])
        nc.sync.dma_start(b_sv[:], bsv1[:])

        ones1 = const_p.tile([1, 128], bf16, tag="ones1")
        nc.vector.memset(ones1[:], 1.0)
        ones_col = const_p.tile([128, 1], bf16, tag="ones_col")
        nc.vector.memset(ones_col[:], 1.0)
        iota_i = const_p.tile([128, 128], i32, tag="iota_i")
        nc.gpsimd.iota(iota_i[:], pattern=[[1, 128]], base=0, channel_multiplier=0)
        iota_f = const_p.tile([128, 128], fp32, tag="iota_f")
        nc.vector.tensor_copy(iota_f[:], iota_i[:])
        from concourse.masks import make_identity
        ident = const_p.tile([128, 128], fp32, tag="ident")
        make_identity(nc, ident[:])

        # persistent SBUF: skip + q rows + edge metadata
        skip_sb = const_p.tile([128, nb, 128], fp32, tag="skip")
        q_sb = const_p.tile([128, nb, 128], bf16, tag="qsb")
        idx_kv_sb = const_p.tile([128, S], i32, tag="ikv")
        dstloc_sb = const_p.tile([128, S], fp32, tag="dl")
        nc.sync.dma_start(idx_kv_sb[:], idx_kv_d[:])
        nc.sync.dma_start(dstloc_sb[:], dstloc_d[:])

        # ---------------- phase 1a: k|v table for all nodes ----------------
        kv_stores = []
        n_full = npad // tw
        tiles1a = [(i * tw, tw) for i in range(n_full)]
        if npad % tw:
            tiles1a.append((n_full * tw, npad % tw))
        with tc.tile_pool(name="p1x", bufs=4) as p1x, \
             tc.tile_pool(name="p1o", bufs=4) as p1o, \
             tc.tile_pool(name="p1ps", bufs=6, space="PSUM") as p1ps:
            for ti, (base, w) in enumerate(tiles1a):
                nj = w // 128
                xt = p1x.tile([128, w], bf16, tag="xt")
                nc.sync.dma_start(xt[:], xT[:, base:base + w])
                kvsb = p1o.tile([128, nj, 256], bf16, tag="kvsb")
                for j0 in range(0, nj, 4):
                    js = list(range(j0, min(j0 + 4, nj)))
                    g = len(js)
                    pk = p1ps.tile([128, g * 128], fp32, tag="ps")
                    pv = p1ps.tile([128, g * 128], fp32, tag="ps")
                    for i, j in enumerate(js):
                        lhs = xt[:, j * 128:(j + 1) * 128]
                        nc.tensor.matmul(out=pk[:, i * 128:(i + 1) * 128],
                                         lhsT=lhs, rhs=w_k[:], start=True, stop=True)
                        nc.tensor.matmul(out=pv[:, i * 128:(i + 1) * 128],
                                         lhsT=lhs, rhs=w_v[:], start=True, stop=True)
                    kv = kvsb[:, j0:j0 + g, :]
                    nc.vector.tensor_copy(kv[:, :, 0:128],
                                          pk[:].rearrange("p (c e) -> p c e", e=128))
                    nc.scalar.activation(kv[:, :, 128:256],
                                         pv[:].rearrange("p (c e) -> p c e", e=128),
                                         Act.Copy)
                out_view = kv_tab[base:base + w, :].rearrange("(j p) e -> p j e", p=128)
                kv_stores.append(nc.sync.dma_start(out_view, kvsb[:]))

        # ---------------- phase 1b: q' and skip for the dst slice ----------------
        n_full_b = dst_pad // tw
        tiles1b = [(i * tw, tw) for i in range(n_full_b)]
        if dst_pad % tw:
            tiles1b.append((n_full_b * tw, dst_pad % tw))
        with tc.tile_pool(name="p2x", bufs=3) as p2x, \
             tc.tile_pool(name="p2o", bufs=2) as p2o, \
             tc.tile_pool(name="p2ps", bufs=4, space="PSUM") as p2ps:
            for (base, w) in tiles1b:
                nj = w // 128
                xt = p2x.tile([128, w], bf16, tag="xst")
                nc.sync.dma_start(xt[:], xTs[:, base:base + w])
                for j in range(nj):
                    lhs = xt[:, j * 128:(j + 1) * 128]
                    blk = base // 128 + j
                    pq = p2ps.tile([128, 128], fp32, tag="ps2")
                    nc.tensor.matmul(out=pq[:], lhsT=lhs, rhs=w_q[:], start=True, stop=False)
                    nc.tensor.matmul(out=pq[:], lhsT=ones1[:], rhs=b_q[:], start=False, stop=True)
                    ps = p2ps.tile([128, 128], fp32, tag="ps2")
                    nc.tensor.matmul(out=ps[:], lhsT=lhs, rhs=w_s[:], start=True, stop=False)
                    nc.tensor.matmul(out=ps[:], lhsT=ones1[:], rhs=b_sv[:], start=False, stop=True)
                    nc.vector.tensor_copy(q_sb[:, blk, :], pq[:])
                    nc.scalar.activation(skip_sb[:, blk, :], ps[:], Act.Copy)

        # ---------------- phase 2: edge attention + scatter ----------------
        from concourse.tile_rust import add_dep_helper
        first_gather = [None]
        with tc.tile_pool(name="gkv", bufs=25) as gkv_p, \
             tc.tile_pool(name="ohp", bufs=45) as oh_p, \
             tc.tile_pool(name="ew", bufs=6) as ew_p, \
             tc.tile_pool(name="epi", bufs=2) as epi_p, \
             tc.tile_pool(name="eps", bufs=2, space="PSUM") as eps_p, \
             tc.tile_pool(name="dps", bufs=2, space="PSUM") as dps_p, \
             tc.tile_pool(name="ops", bufs=2, space="PSUM") as ops_p, \
             tc.tile_pool(name="qps", bufs=2, space="PSUM") as qps_p:
            for b in range(nb):
                cmax_b = cc[b]
                cb = colbase[b]
                pairs = []
                c0 = 0
                while c0 < cmax_b:
                    pairs.append(tuple(range(c0, min(c0 + 2, cmax_b))))
                    c0 += 2
                logit_blk = ew_p.tile([128, cmax_b], fp32, tag="lb")
                ohs = {}
                kvgs = {}
                # pass A: gather, one-hot, Qg via PE, logits
                for chunks in pairs:
                    w = len(chunks)
                    kvg = gkv_p.tile([128, w * 256], bf16, tag="kvg")
                    for i, c in enumerate(chunks):
                        col = cb + c
                        gi = nc.gpsimd.indirect_dma_start(
                            out=kvg[:, i * 256:(i + 1) * 256], out_offset=None,
                            in_=kv_tab[:],
                            in_offset=bass.IndirectOffsetOnAxis(
                                ap=idx_kv_sb[:, col:col + 1], axis=0))
                        if first_gather[0] is None:
                            first_gather[0] = gi
                            for s in kv_stores:
                                add_dep_helper(gi.ins, s.ins, reason="kv_tab raw")
                    kvgs[chunks[0]] = kvg
                    pot = ops_p.tile([128, w * 128], fp32, tag="pot")
                    for i, c in enumerate(chunks):
                        col = cb + c
                        oh = oh_p.tile([128, 128], fp32, tag="oh")
                        nc.vector.tensor_scalar(
                            out=oh[:], in0=iota_f[:],
                            scalar1=dstloc_sb[:, col:col + 1],
                            scalar2=None, op0=Alu.is_equal)
                        ohs[c] = oh
                        nc.tensor.transpose(out=pot[:, i * 128:(i + 1) * 128],
                                            in_=oh[:], identity=ident[:])
                    ot = ew_p.tile([128, w * 128], bf16, tag="ot")
                    nc.scalar.activation(ot[:], pot[:], Act.Copy)
                    pqg = qps_p.tile([128, w * 128], fp32, tag="pqg")
                    for i in range(w):
                        nc.tensor.matmul(out=pqg[:, i * 128:(i + 1) * 128],
                                         lhsT=ot[:, i * 128:(i + 1) * 128],
                                         rhs=q_sb[:, b, :], start=True, stop=True)
                    prod = ew_p.tile([128, w * 128], bf16, tag="prod")
                    kview = kvg[:].rearrange("p (c x) -> p c x", x=256)[:, :, 0:128]
                    nc.vector.tensor_tensor(
                        out=prod[:].rearrange("p (c e) -> p c e", e=128),
                        in0=pqg[:].rearrange("p (c e) -> p c e", e=128),
                        in1=kview, op=Alu.mult)
                    nc.vector.reduce_sum(
                        out=logit_blk[:, chunks[0]:chunks[0] + w],
                        in_=prod[:].rearrange("p (c e) -> p c e", e=128),
                        axis=mybir.AxisListType.X)
                ex_blk = ew_p.tile([128, cmax_b], fp32, tag="exb")
                nc.scalar.activation(ex_blk[:], logit_blk[:], Act.Exp)
                # pass B: weighted one-hot scatter via PE
                pagg = eps_p.tile([128, 128], fp32, tag="pagg")
                pden = dps_p.tile([128, 1], fp32, tag="pden")
                for c in range(cmax_b):
                    mex = ew_p.tile([128, 128], bf16, tag="mex")
                    nc.scalar.activation(mex[:], ohs[c][:], Act.Copy,
                                         scale=ex_blk[:, c:c + 1])
                    kvg = kvgs[(c // 2) * 2]
                    i = c % 2
                    vslc = kvg[:, i * 256 + 128:i * 256 + 256]
                    nc.tensor.matmul(out=pagg[:], lhsT=mex[:], rhs=vslc,
                                     start=(c == 0), stop=(c == cmax_b - 1))
                    nc.tensor.matmul(out=pden[:], lhsT=mex[:], rhs=ones_col[:],
                                     start=(c == 0), stop=(c == cmax_b - 1))
                if True:
                    den = epi_p.tile([128, 1], fp32, tag="den")
                    nc.vector.tensor_scalar_add(den[:], pden[:], 1e-30)
                    rec = epi_p.tile([128, 1], fp32, tag="rec")
                    nc.vector.reciprocal(rec[:], den[:])
                    z = epi_p.tile([128, 128], fp32, tag="z")
                    nc.scalar.activation(z[:], pagg[:], Act.Copy, scale=rec[:])
                    z2 = epi_p.tile([128, 128], fp32, tag="z2")
                    nc.vector.tensor_tensor(out=z2[:], in0=z[:], in1=skip_sb[:, b, :], op=Alu.add)
                    zn = epi_p.tile([128, 128], fp32, tag="zn")
                    nc.vector.tensor_scalar_min(zn[:], z2[:], 0.0)
                    en = epi_p.tile([128, 128], fp32, tag="en")
                    nc.scalar.activation(en[:], zn[:], Act.Exp)
                    zp = epi_p.tile([128, 128], fp32, tag="zp")
                    nc.scalar.activation(zp[:], z2[:], Act.Relu)
                    o1 = epi_p.tile([128, 128], fp32, tag="o1")
                    nc.vector.tensor_tensor(out=o1[:], in0=en[:], in1=zp[:], op=Alu.add)
                    o2 = epi_p.tile([128, 128], fp32, tag="o2")
                    nc.vector.tensor_scalar_add(o2[:], o1[:], -1.0)
                    nc.sync.dma_start(out_d[b * 128:(b + 1) * 128, :], o2[:])

    nc.compile()
    return nc


_NC_CACHE = {}


def _get_nc(cc):
    if cc not in _NC_CACHE:
        _NC_CACHE[cc] = _build_nc(cc)
    return _NC_CACHE[cc]


def _make_in_maps(inputs, idx_kv, dstloc, perm):
    x = np.asarray(inputs["x"], np.float32)
    xb = x.astype(BF16)
    xT_full = np.zeros((128, NPAD), BF16)
    xT_full[:, :N] = xb.T
    wq = (np.asarray(inputs["Wq"], np.float32) * SCALE).astype(BF16)
    wk = np.asarray(inputs["Wk"], np.float32).astype(BF16)
    wv = np.asarray(inputs["Wv"], np.float32).astype(BF16)
    ws = np.asarray(inputs["Ws"], np.float32).astype(BF16)
    bq1 = (np.asarray(inputs["bq"], np.float32) * SCALE).astype(BF16).reshape(1, 128)
    bsv1 = (np.asarray(inputs["bs"], np.float32)
            + np.asarray(inputs["bv"], np.float32)).astype(BF16).reshape(1, 128)

    in_maps = []
    for c in range(M_CORES):
        xs_local = np.zeros((DST_PAD, 128), BF16)
        xs_local[:DPC] = xb[c * DPC:(c + 1) * DPC]
        xTs = xs_local[np.minimum(perm[c], DST_PAD - 1)].T.copy()
        in_maps.append({
            "xT": xT_full, "xTs": xTs,
            "Wq": wq, "Wk": wk, "Wv": wv, "Ws": ws,
            "bq1": bq1, "bsv1": bsv1,
            "idx_kv": idx_kv[c], "dstloc": dstloc[c],
        })
    return in_maps


def kernel(x, edge_index, Wq, bq, Wk, bk, Wv, bv, Ws, bs):
    from concourse import bass_utils

    idx_kv, dstloc, perm, cc = _host_prep(edge_index)
    in_maps = _make_in_maps(
        {"x": x, "Wq": Wq, "Wk": Wk, "Wv": Wv, "Ws": Ws,
         "bq": bq, "bs": bs, "bv": bv}, idx_kv, dstloc, perm)
    nc = _get_nc(cc)
    res = bass_utils.run_bass_kernel_spmd(nc, in_maps, core_ids=list(range(M_CORES)))
    out = np.zeros((N, 128), np.float32)
    for c in range(M_CORES):
        rows = res.results[c]["out"]          # [DST_PAD, 128] in device order
        p = perm[c]
        valid = p < DPC
        out[c * DPC + p[valid]] = rows[valid]
    return out

